# revision 1
# baseline (speedup 1.0000x reference)
"""Bass/Trainium2 kernel for nn_CharLevelLanguageModel (6-layer char transformer).

Strategy: data-parallel over batch (64 -> 8 cores x 8). Full forward in one NEFF
per core, emitted as a software pipeline over 24 iterations (6 layers x 4
batch-pairs; each layer is fully local to a 512-token batch-pair). Emission
order per step is  A(i+1) | Wo+LN2(i) | ATT(i+1) | FFN(i)  so every serial
dependency chain (LN row ops, softmax normalize) is covered by another
iteration's matmul work and the PE stays busy (warm HAM clock).

Activations are feature-major f32r. LayerNorm affine params are folded into
adjacent weights on the host; on-device LN is standardization via ones-matmul
stats + row chain + gpsimd partition broadcasts. Attention is key-major:
softmax needs no transposes; sums ride a ones-column appended to V; causal
masking is one multiplicative gpsimd multiply per (head, batch).
"""

import os
import numpy as np

import concourse.bass as bass
import concourse.mybir as mybir
import concourse.tile as tile
from concourse import bacc
from concourse.bass_utils import run_bass_kernel_spmd

B, T, C, H, L, V = 64, 256, 384, 6, 6, 65
HS = C // H          # 64
DFF = 4 * C          # 1536
N_CORES = 8
BPC = B // N_CORES   # 8 batches per core
NTOK = BPC * T       # 2048 tokens per core
NBP = 4              # batch-pair (512-token) tiles per core
KC = C // 128        # 3 feature chunks
K12 = DFF // 128     # 12 dff chunks
EPS = 1e-5
SCALE = HS ** -0.5

f32 = mybir.dt.float32
f32r = mybir.dt.float32r
AF = mybir.ActivationFunctionType
ALU = mybir.AluOpType

N_LAYERS = int(os.environ.get("KERNEL_LAYERS", str(L)))

_cache = {}


def _build_nc():
    nc = bacc.Bacc("TRN2", target_bir_lowering=False, debug=False,
                   num_devices=N_CORES)

    x0T_d = nc.dram_tensor("x0T", [C, NTOK], f32r, kind="ExternalInput").ap()
    wqkv_d = nc.dram_tensor("wqkv", [L, C, 3 * C], f32r, kind="ExternalInput").ap()
    bqkv_d = nc.dram_tensor("bqkv", [L, 128, 6], f32, kind="ExternalInput").ap()
    wo_d = nc.dram_tensor("wo", [L, C, C], f32r, kind="ExternalInput").ap()
    w1_d = nc.dram_tensor("w1", [L, C, DFF], f32r, kind="ExternalInput").ap()
    b1_d = nc.dram_tensor("b1", [L, 128, K12], f32, kind="ExternalInput").ap()
    w2_d = nc.dram_tensor("w2", [L, DFF, C], f32r, kind="ExternalInput").ap()
    brows_d = nc.dram_tensor("brows", [L, 1, 2 * C], f32r, kind="ExternalInput").ap()
    wlm_d = nc.dram_tensor("wlm", [C, V], f32r, kind="ExternalInput").ap()
    blm_d = nc.dram_tensor("blm", [V], f32, kind="ExternalInput").ap()
    m01_d = nc.dram_tensor("m01", [128, 512], f32, kind="ExternalInput").ap()
    outT_d = nc.dram_tensor("outT", [V, NTOK], f32, kind="ExternalOutput").ap()

    with tile.TileContext(nc) as tc:
        _build_body(nc, tc, x0T_d, wqkv_d, bqkv_d, wo_d, w1_d, b1_d, w2_d,
                    brows_d, wlm_d, blm_d, m01_d, outT_d)
    nc.compile()
    return nc


def _build_body(nc, tc, x0T_d, wqkv_d, bqkv_d, wo_d, w1_d, b1_d, w2_d,
                brows_d, wlm_d, blm_d, m01_d, outT_d):
    import contextlib
    ctx = contextlib.ExitStack()
    p_const = ctx.enter_context(tc.tile_pool(name="consts", bufs=1))
    p_x = ctx.enter_context(tc.tile_pool(name="x", bufs=1))
    p_w = ctx.enter_context(tc.tile_pool(name="w", bufs=1))
    p_xn = ctx.enter_context(tc.tile_pool(name="xn", bufs=2))
    p_xsq = ctx.enter_context(tc.tile_pool(name="xsq", bufs=1))
    p_rows = ctx.enter_context(tc.tile_pool(name="rows", bufs=1))
    p_bc = ctx.enter_context(tc.tile_pool(name="bc", bufs=1))
    p_qk = ctx.enter_context(tc.tile_pool(name="qk", bufs=2))
    p_v = ctx.enter_context(tc.tile_pool(name="v", bufs=1))
    p_e = ctx.enter_context(tc.tile_pool(name="e", bufs=2))
    p_sm = ctx.enter_context(tc.tile_pool(name="sm", bufs=2))
    p_attc = ctx.enter_context(tc.tile_pool(name="attc", bufs=2))
    p_a = ctx.enter_context(tc.tile_pool(name="a", bufs=3))
    p_out = ctx.enter_context(tc.tile_pool(name="out", bufs=1))
    ps_ap = ctx.enter_context(tc.tile_pool(name="ps_ap", bufs=2, space="PSUM"))
    ps_big = ctx.enter_context(tc.tile_pool(name="ps_big", bufs=3, space="PSUM"))
    ps_fp2 = ctx.enter_context(tc.tile_pool(name="ps_fp2", bufs=1, space="PSUM"))

    # ---- constants ----
    stage = p_const.tile([128, 8], f32, tag="stage")
    onesC = p_const.tile([128, 2], f32r, tag="onesC")      # 1/C for mean matmuls
    nc.vector.memset(stage[:, 0:2], 1.0 / C)
    nc.vector.tensor_copy(onesC[:], stage[:, 0:2])
    onesH = p_const.tile([128, H], f32r, tag="onesH")      # ones col for V_ext
    nc.vector.memset(stage[:, 2:2 + H], 1.0)
    nc.vector.tensor_copy(onesH[:], stage[:, 2:2 + H])
    stage_row = p_const.tile([1, 512], f32, tag="stage_row")
    nc.vector.memset(stage_row, 1.0)
    onesrow = p_const.tile([1, 512], f32r, tag="onesrow")  # moving row for folds
    nc.vector.tensor_copy(onesrow[:], stage_row[:])
    eps_t = p_const.tile([1, 1], f32, tag="eps")
    nc.vector.memset(eps_t, EPS)
    m01 = p_const.tile([128, 512], f32, tag="m01")
    nc.sync.dma_start(out=m01, in_=m01_d)
    blm_t = p_const.tile([V, 1], f32, tag="blm")
    nc.sync.dma_start(out=blm_t, in_=blm_d.rearrange("(v o) -> v o", o=1))
    wlm_t = [p_const.tile([128, V], f32r, tag=f"wlm{kc}", name=f"wlm{kc}")
             for kc in range(KC)]
    for kc in range(KC):
        nc.sync.dma_start(out=wlm_t[kc], in_=wlm_d[kc * 128:(kc + 1) * 128, :])

    # ---- residual stream ----
    x_t = [[p_x.tile([128, 512], f32r, tag=f"x{kc}_{nt}", name=f"x{kc}_{nt}")
            for nt in range(NBP)] for kc in range(KC)]
    for kc in range(KC):
        for nt in range(NBP):
            nc.sync.dma_start(out=x_t[kc][nt],
                              in_=x0T_d[kc * 128:(kc + 1) * 128,
                                        nt * 512:nt * 512 + 512])

    weights = {}

    def load_wqkv(l):
        w = weights.setdefault(l, {})
        w["wqkv"] = [p_w.tile([128, 3 * C], f32r, tag=f"wqkv{kc}",
                              name=f"wqkv{kc}", bufs=2) for kc in range(KC)]
        for kc in range(KC):
            nc.sync.dma_start(out=w["wqkv"][kc],
                              in_=wqkv_d[l, kc * 128:(kc + 1) * 128, :])
        w["bqkv"] = p_w.tile([128, 6], f32, tag="bqkv", name="bqkv", bufs=2)
        nc.sync.dma_start(out=w["bqkv"], in_=bqkv_d[l])

    def load_rest(l):
        w = weights.setdefault(l, {})
        w["wo"] = [p_w.tile([128, C], f32r, tag=f"wo{kc}", name=f"wo{kc}")
                   for kc in range(KC)]
        for kc in range(KC):
            nc.sync.dma_start(out=w["wo"][kc],
                              in_=wo_d[l, kc * 128:(kc + 1) * 128, :])
        w["w1"] = [p_w.tile([128, DFF], f32r, tag=f"w1{kc}", name=f"w1{kc}")
                   for kc in range(KC)]
        for kc in range(KC):
            nc.sync.dma_start(out=w["w1"][kc],
                              in_=w1_d[l, kc * 128:(kc + 1) * 128, :])
        w["b1"] = p_w.tile([128, K12], f32, tag="b1", name="b1")
        nc.sync.dma_start(out=w["b1"], in_=b1_d[l])
        w["w2"] = [p_w.tile([128, C], f32r, tag=f"w2_{kc}", name=f"w2_{kc}")
                   for kc in range(K12)]
        for kc in range(K12):
            nc.sync.dma_start(out=w["w2"][kc],
                              in_=w2_d[l, kc * 128:(kc + 1) * 128, :])
        w["brows"] = p_w.tile([1, 2 * C], f32r, tag="brows", name="brows")
        nc.sync.dma_start(out=w["brows"], in_=brows_d[l])

    def ln_block(nt, tagp):
        """Standardize x_t[:, nt]: returns list of 3 [128,512] f32r tiles."""
        mu_t = ps_ap.tile([2, 512], f32, tag="ap", name="mu_t")
        sq_t = ps_ap.tile([2, 512], f32, tag="ap", name="sq_t")
        for kc in range(KC):
            nc.tensor.matmul(mu_t[0:2, :], onesC[:], x_t[kc][nt][:],
                             start=(kc == 0), stop=(kc == KC - 1))
        for kc in range(KC):
            xsq = p_xsq.tile([128, 512], f32r, tag="xsq", name="xsq")
            nc.gpsimd.tensor_mul(xsq[:], x_t[kc][nt][:], x_t[kc][nt][:])
            nc.tensor.matmul(sq_t[0:2, :], onesC[:], xsq[:],
                             start=(kc == 0), stop=(kc == KC - 1))
        rows = p_rows.tile([1, 2 * 512], f32, tag="lnrows", name="lnrows")
        Br, Cr = rows[:, 0:512], rows[:, 512:1024]
        nc.scalar.square(Cr, mu_t[0:1, :])
        nc.vector.tensor_tensor(out=Br, in0=sq_t[0:1, :], in1=Cr,
                                op=ALU.subtract)
        nc.scalar.activation(Br, Br, AF.Sqrt, bias=eps_t[:], scale=1.0)
        nc.vector.reciprocal_approx_fast(out=Br, in_=Br)      # rs
        nc.vector.tensor_mul(Cr, mu_t[0:1, :], Br)            # mr = mu*rs
        rs_b = p_bc.tile([128, 512], f32, tag="rs_b", name="rs_b")
        mr_b = p_bc.tile([128, 512], f32, tag="mr_b", name="mr_b")
        nc.gpsimd.partition_broadcast(rs_b[:], Br)
        nc.gpsimd.partition_broadcast(mr_b[:], Cr)
        outs = []
        for kc in range(KC):
            xs = p_xsq.tile([128, 512], f32, tag="xs", name="xs")
            nc.gpsimd.tensor_mul(xs[:], x_t[kc][nt][:], rs_b[:])
            o = p_xn.tile([128, 512], f32r, tag=f"{tagp}{kc}", name=f"{tagp}{kc}")
            nc.vector.tensor_tensor(out=o[:], in0=xs[:], in1=mr_b[:],
                                    op=ALU.subtract)
            outs.append(o)
        return outs

    state = {}

    def emit_LN1(it):
        l, bp = divmod(it, NBP)
        state[it] = {"xn": ln_block(bp, "xn")}

    def emit_QV(it):
        l, bp = divmod(it, NBP)
        w = weights[l]
        xn = state[it]["xn"]
        qk = []
        for oc in range(6):
            qp = ps_big.tile([128, 512], f32, tag="big", name="qp")
            for kc in range(KC):
                nc.tensor.matmul(qp[:], w["wqkv"][kc][:, oc * 128:oc * 128 + 128],
                                 xn[kc][:], start=(kc == 0), stop=(kc == KC - 1))
            qt = p_qk.tile([128, 512], f32r, tag=f"qk{oc}", name=f"qk{oc}")
            nc.scalar.activation(qt[:], qp[:], AF.Identity,
                                 bias=w["bqkv"][:, oc:oc + 1], scale=1.0)
            qk.append(qt)
        vext = []
        for bi in range(2):
            vx = p_v.tile([128, 2 * H * (HS + 1)], f32r, tag=f"vext{bi}",
                          name=f"vext{bi}")
            vxr = vx.rearrange("p (j h e) -> p j h e", j=2, h=H)
            for j in range(2):
                vp = ps_big.tile([128, C], f32, tag="big", name="vp")
                tc0 = bi * 256 + j * 128
                for kc in range(KC):
                    nc.tensor.matmul(vp[:], xn[kc][:, tc0:tc0 + 128],
                                     w["wqkv"][kc][:, 2 * C:3 * C],
                                     start=(kc == 0), stop=(kc == KC - 1))
                nc.vector.tensor_copy(vxr[:, j, :, 0:HS],
                                      vp[:].rearrange("p (h d) -> p h d", h=H))
                nc.gpsimd.tensor_copy(out=vxr[:, j, :, HS:HS + 1], in_=onesH[:])
            vext.append(vx)
        state[it]["qk"] = qk
        state[it]["vext"] = vext

    def emit_ATT(it):
        """Wave-pipelined: attV lags scores by 2 units so the PE never sits
        right behind the exp/mask chain of the unit it just issued."""
        st = state[it]
        qk, vext = st["qk"], st["vext"]
        attc = [p_attc.tile([128, 512], f32r, tag=f"attc{kc}", name=f"attc{kc}")
                for kc in range(KC)]
        ap_t = {}
        e_ms = {}
        LAG = 2
        for u in range(12 + LAG):
            if u < 12:
                h, bi = divmod(u, 2)
                qch, kch = h // 2, 3 + h // 2
                qrow = (h % 2) * 64
                q0 = bi * 256
                sp = ps_big.tile([128, 512], f32, tag="big", name="sp")
                qs = qk[qch][qrow:qrow + 64, q0:q0 + 256]
                nc.tensor.matmul(sp[:, 0:256],
                                 qk[kch][qrow:qrow + 64, q0:q0 + 128],
                                 qs, start=True, stop=True)
                nc.tensor.matmul(sp[:, 256:512],
                                 qk[kch][qrow:qrow + 64, q0 + 128:q0 + 256],
                                 qs, start=True, stop=True)
                e_t = p_e.tile([128, 512], f32, tag="e_t", name="e_t")
                nc.scalar.activation(e_t[:], sp[:], AF.Exp, bias=0.0,
                                     scale=SCALE)
                e_m = p_e.tile([128, 512], f32r, tag="e_m", name="e_m", bufs=4)
                nc.gpsimd.tensor_mul(e_m[:], e_t[:], m01[:])
                e_ms[u] = e_m
            if u >= LAG:
                v = u - LAG
                h, bi = divmod(v, 2)
                qch = h // 2
                qrow = (h % 2) * 64
                q0 = bi * 256
                if bi == 0:
                    ap_t[h] = ps_ap.tile([HS + 1, 512], f32, tag="ap", name="ap_")
                ap_ = ap_t[h]
                e_m = e_ms.pop(v)
                vxr = vext[bi].rearrange("p (j h e) -> p j h e", j=2, h=H)
                nc.tensor.matmul(ap_[:, q0:q0 + 256], vxr[:, 0, h, :],
                                 e_m[:, 0:256], start=True, stop=False)
                nc.tensor.matmul(ap_[:, q0:q0 + 256], vxr[:, 1, h, :],
                                 e_m[:, 256:512], start=False, stop=True)
                if bi == 1:
                    srow = p_sm.tile([1, 512], f32, tag="srow", name="srow",
                                     bufs=1)
                    nc.scalar.copy(srow[:], ap_[HS:HS + 1, :])
                    rec = p_sm.tile([1, 512], f32, tag="rec", name="rec",
                                    bufs=1)
                    nc.vector.reciprocal_approx_fast(out=rec[:], in_=srow[:])
                    r_b = p_sm.tile([64, 512], f32, tag="r_b", name="r_b")
                    nc.gpsimd.partition_broadcast(r_b[:], rec[:])
                    nc.vector.tensor_mul(attc[qch][qrow:qrow + 64, :],
                                         ap_[0:HS, :], r_b[:])
        state[it]["attc"] = attc
        del state[it]["qk"], state[it]["vext"], state[it]["xn"]

    def emit_Wo(it):
        l, bp = divmod(it, NBP)
        w = weights[l]
        attc = state[it]["attc"]
        for oc in range(KC):
            wp = ps_big.tile([128, 512], f32, tag="big", name="wp")
            nc.tensor.matmul(wp[:], w["brows"][0:1, oc * 128:oc * 128 + 128],
                             onesrow[:], start=True, stop=False)
            for kc in range(KC):
                nc.tensor.matmul(wp[:], w["wo"][kc][:, oc * 128:oc * 128 + 128],
                                 attc[kc][:], start=False, stop=(kc == KC - 1))
            nc.vector.tensor_tensor(out=x_t[oc][bp][:], in0=wp[:],
                                    in1=x_t[oc][bp][:], op=ALU.add)
        del state[it]["attc"]

    def emit_LN2(it):
        l, bp = divmod(it, NBP)
        state[it]["h2n"] = ln_block(bp, "h2n")

    def emit_FFN(it):
        l, bp = divmod(it, NBP)
        w = weights[l]
        h2n = state[it]["h2n"]
        fp2 = [ps_fp2.tile([128, 512], f32, tag=f"fp2_{oc}", name=f"fp2_{oc}")
               for oc in range(KC)]
        for oc in range(KC):
            nc.tensor.matmul(fp2[oc][:],
                             w["brows"][0:1, C + oc * 128:C + oc * 128 + 128],
                             onesrow[:], start=True, stop=False)
        for kc12 in range(K12):
            fp1 = ps_big.tile([128, 512], f32, tag="big", name="fp1")
            for kc in range(KC):
                nc.tensor.matmul(fp1[:],
                                 w["w1"][kc][:, kc12 * 128:kc12 * 128 + 128],
                                 h2n[kc][:], start=(kc == 0), stop=(kc == KC - 1))
            a = p_a.tile([128, 512], f32r, tag="a", name="a")
            nc.scalar.activation(a[:], fp1[:], AF.Relu,
                                 bias=w["b1"][:, kc12:kc12 + 1], scale=1.0)
            for oc in range(KC):
                nc.tensor.matmul(fp2[oc][:],
                                 w["w2"][kc12][:, oc * 128:oc * 128 + 128],
                                 a[:], start=False, stop=(kc12 == K12 - 1))
        for oc in range(KC):
            nc.vector.tensor_tensor(out=x_t[oc][bp][:], in0=fp2[oc][:],
                                    in1=x_t[oc][bp][:], op=ALU.add)
        del state[it]

    # ---- pipelined emission ----
    # step i: QV(i) | FFN(i-1) | ATT(i) | LN1(i+1) | Wo(i) | LN2(i)
    # Every serial chain (LN row chain, softmax normalize) is followed in
    # each engine queue by independent work from a neighboring iteration.
    NITER = N_LAYERS * NBP
    load_wqkv(0)
    load_rest(0)
    if NITER > 0:
        emit_LN1(0)
    for it in range(NITER + 1):
        if it < NITER:
            l, bp = divmod(it, NBP)
            if bp == 2 and l + 1 < N_LAYERS:
                load_wqkv(l + 1)
            emit_QV(it)
        if it >= 1:
            emit_FFN(it - 1)
            pl, pbp = divmod(it - 1, NBP)
            if pbp == NBP - 1 and pl + 1 < N_LAYERS:
                load_rest(pl + 1)
        if it < NITER:
            emit_ATT(it)
            if it + 1 < NITER:
                emit_LN1(it + 1)
            emit_Wo(it)
            emit_LN2(it)

    # ---- final LN + LM head ----
    for nt in range(NBP):
        xf = ln_block(nt, "xn")
        lp = ps_big.tile([V, 512], f32, tag="big", name="lp")
        for kc in range(KC):
            nc.tensor.matmul(lp[:], wlm_t[kc][:], xf[kc][:],
                             start=(kc == 0), stop=(kc == KC - 1))
        osb = p_out.tile([V, 512], f32, tag="osb", name="osb")
        nc.scalar.activation(osb[:], lp[:], AF.Identity, bias=blm_t[:],
                             scale=1.0)
        nc.sync.dma_start(out=outT_d[:, nt * 512:nt * 512 + 512], in_=osb[:])

    ctx.close()


def _host_prep(inputs):
    """Fold LN affine params into weights; build per-core input maps."""
    f = lambda k: np.asarray(inputs[k], dtype=np.float32)
    idx = np.asarray(inputs["idx"]).astype(np.int64)
    tok_emb, pos_emb = f("tok_emb"), f("pos_emb")
    Wq, Wk, Wv, Wo = f("Wq"), f("Wk"), f("Wv"), f("Wo")
    bo, W1, b1, W2, b2 = f("bo"), f("W1"), f("b1"), f("W2"), f("b2")
    ln1_g, ln1_b = f("ln1_g"), f("ln1_b")
    ln2_g, ln2_b = f("ln2_g"), f("ln2_b")
    lnf_g, lnf_b = f("lnf_g"), f("lnf_b")
    Wlm, blm = f("Wlm"), f("blm")

    # [L,H,C,HS] -> [L,C,H*HS]
    Wq_all = np.transpose(Wq, (0, 2, 1, 3)).reshape(L, C, C)
    Wk_all = np.transpose(Wk, (0, 2, 1, 3)).reshape(L, C, C)
    Wv_all = np.transpose(Wv, (0, 2, 1, 3)).reshape(L, C, C)

    g1 = ln1_g[:, :, None]
    wqkv = np.concatenate([g1 * Wq_all, g1 * Wk_all, g1 * Wv_all], axis=2)
    bq = np.einsum("lc,lcd->ld", ln1_b, Wq_all)
    bk = np.einsum("lc,lcd->ld", ln1_b, Wk_all)
    bv = np.einsum("lc,lcd->ld", ln1_b, Wv_all)
    bo2 = bo + np.einsum("ld,ldc->lc", bv, Wo)       # v-bias folds through Wo
    w1f = ln2_g[:, :, None] * W1
    b1f = b1 + np.einsum("lc,lcd->ld", ln2_b, W1)
    wlmf = lnf_g[:, None] * Wlm
    blmf = blm + lnf_b @ Wlm

    bqkv = np.concatenate([bq, bk], axis=1)          # [L, 768]
    bqkv_cols = np.ascontiguousarray(
        bqkv.reshape(L, 6, 128).transpose(0, 2, 1))  # [L,128,6]
    b1_cols = np.ascontiguousarray(
        b1f.reshape(L, K12, 128).transpose(0, 2, 1))  # [L,128,12]
    brows = np.ascontiguousarray(
        np.concatenate([bo2, b2], axis=1)[:, None, :])  # [L,1,2C]

    # multiplicative causal mask, key-major: cols = (key_block, q)
    p = np.arange(128)[:, None]
    q = np.arange(256)[None, :]
    m0 = (p <= q).astype(np.float32)          # keys 0..127
    m1 = (p + 128 <= q).astype(np.float32)    # keys 128..255
    m01 = np.concatenate([m0, m1], axis=1)    # [128, 512]

    x0 = tok_emb[idx] + pos_emb[None]                # [B,T,C] f32
    in_maps = []
    for c in range(N_CORES):
        x0c = x0[c * BPC:(c + 1) * BPC].reshape(NTOK, C)
        in_maps.append({
            "x0T": np.ascontiguousarray(x0c.T),
            "wqkv": np.ascontiguousarray(wqkv),
            "bqkv": bqkv_cols,
            "wo": np.ascontiguousarray(Wo),
            "w1": np.ascontiguousarray(w1f),
            "b1": b1_cols,
            "w2": np.ascontiguousarray(W2),
            "brows": brows,
            "wlm": np.ascontiguousarray(wlmf),
            "blm": np.ascontiguousarray(blmf),
            "m01": m01,
        })
    return in_maps


def _run(inputs, trace=False):
    if "nc" not in _cache:
        _cache["nc"] = _build_nc()
    nc = _cache["nc"]
    in_maps = _host_prep(inputs)
    res = run_bass_kernel_spmd(nc, in_maps, core_ids=list(range(N_CORES)),
                               trace=trace)
    outs = []
    for c in range(N_CORES):
        outT = res.results[c]["outT"]                 # [V, NTOK]
        outs.append(outT.T.reshape(BPC, T, V))
    logits = np.concatenate(outs, axis=0).astype(np.float32)
    return logits, res


def kernel(**inputs) -> np.ndarray:
    logits, _ = _run(inputs, trace=False)
    return logits



# revision 12
# speedup vs baseline: 2.0462x; 2.0462x over previous
"""Bass/Trainium2 kernel for nn_CharLevelLanguageModel (6-layer char transformer).

v2 strategy: data-parallel over batch (64 -> 8 cores x 8). Per core, each layer
is emitted stage-major across the 4 batch-pair (512-token) tiles so that every
serial chain (LN row ops, softmax normalize) is covered by independent matmul
work from the other batch-pairs.

Key choices vs v1:
- bf16 matmul operands (weights + activations); residual stream and stats stay
  fp32 (f32r) in SBUF; PSUM accumulates fp32.
- Zero GpSimd instructions. Partition broadcasts are ones-column matmuls on the
  PE; elementwise work is split between DVE and ACT.
- Single ACT table set ("natural_log_exp_and_others"): rsqrt(v)=exp(-0.5*ln v),
  1/s = exp(-ln s). No table reloads after startup.
- LN affine params folded into adjacent weights on host; biases ride K=1
  bf16 matmul rows (brows x onesrow) or ACT bias columns.
- Causal mask is one multiplicative bf16 DVE op per (head, batch) unit.
"""

import os
import numpy as np
import ml_dtypes

import concourse.bass as bass
import concourse.mybir as mybir
import concourse.tile as tile
from concourse import bacc
from concourse.bass_utils import run_bass_kernel_spmd

B, T, C, H, L, V = 64, 256, 384, 6, 6, 65
HS = C // H          # 64
DFF = 4 * C          # 1536
N_CORES = 8
BPC = B // N_CORES   # 8 batches per core
NTOK = BPC * T       # 2048 tokens per core
NBP = 4              # batch-pair (512-token) tiles per core
KC = C // 128        # 3 feature chunks
K12 = DFF // 128     # 12 dff chunks
EPS = 1e-5
SCALE = HS ** -0.5

f32 = mybir.dt.float32
f32r = mybir.dt.float32r
bf16 = mybir.dt.bfloat16
AF = mybir.ActivationFunctionType
ALU = mybir.AluOpType

N_LAYERS = int(os.environ.get("KERNEL_LAYERS", str(L)))

_cache = {}


def _build_nc():
    nc = bacc.Bacc("TRN2", target_bir_lowering=False, debug=False,
                   num_devices=N_CORES)

    x0T_d = nc.dram_tensor("x0T", [C, NTOK], f32r, kind="ExternalInput").ap()
    wqkv_d = nc.dram_tensor("wqkv", [L, C, 3 * C], bf16, kind="ExternalInput").ap()
    bqkv_d = nc.dram_tensor("bqkv", [L, 128, 6], f32, kind="ExternalInput").ap()
    wo_d = nc.dram_tensor("wo", [L, C, C], bf16, kind="ExternalInput").ap()
    w1_d = nc.dram_tensor("w1", [L, C, DFF], bf16, kind="ExternalInput").ap()
    b1_d = nc.dram_tensor("b1", [L, 128, K12], f32, kind="ExternalInput").ap()
    w2_d = nc.dram_tensor("w2", [L, DFF, C], bf16, kind="ExternalInput").ap()
    brows_d = nc.dram_tensor("brows", [L, 1, 2 * C], bf16, kind="ExternalInput").ap()
    wlm_d = nc.dram_tensor("wlm", [C, V], bf16, kind="ExternalInput").ap()
    blm_d = nc.dram_tensor("blm", [V], f32, kind="ExternalInput").ap()
    m01_d = nc.dram_tensor("m01", [128, 512], bf16, kind="ExternalInput").ap()
    outT_d = nc.dram_tensor("outT", [V, NTOK], f32, kind="ExternalOutput").ap()

    with tile.TileContext(nc) as tc:
        _build_body(nc, tc, x0T_d, wqkv_d, bqkv_d, wo_d, w1_d, b1_d, w2_d,
                    brows_d, wlm_d, blm_d, m01_d, outT_d)
    nc.compile()
    return nc


def _build_body(nc, tc, x0T_d, wqkv_d, bqkv_d, wo_d, w1_d, b1_d, w2_d,
                brows_d, wlm_d, blm_d, m01_d, outT_d):
    import contextlib
    ctx = contextlib.ExitStack()
    p_const = ctx.enter_context(tc.tile_pool(name="consts", bufs=1))
    p_x = ctx.enter_context(tc.tile_pool(name="x", bufs=1))
    p_w = ctx.enter_context(tc.tile_pool(name="w", bufs=1))
    p_big = ctx.enter_context(tc.tile_pool(name="bigsb", bufs=1))   # xsq/xc
    p_xn = ctx.enter_context(tc.tile_pool(name="xn", bufs=1))
    p_rows = ctx.enter_context(tc.tile_pool(name="rows", bufs=1))
    p_qk = ctx.enter_context(tc.tile_pool(name="qk", bufs=1))
    p_v = ctx.enter_context(tc.tile_pool(name="v", bufs=1))
    p_e = ctx.enter_context(tc.tile_pool(name="e", bufs=1))
    p_attc = ctx.enter_context(tc.tile_pool(name="attc", bufs=1))
    p_a = ctx.enter_context(tc.tile_pool(name="a", bufs=1))
    p_out = ctx.enter_context(tc.tile_pool(name="out", bufs=2))
    ps_aux = ctx.enter_context(tc.tile_pool(name="ps_aux", bufs=2, space="PSUM"))
    ps_ap = ctx.enter_context(tc.tile_pool(name="ps_ap", bufs=2, space="PSUM"))
    ps_big = ctx.enter_context(tc.tile_pool(name="ps_big", bufs=4, space="PSUM"))

    # ---- constants ----
    stage = p_const.tile([128, 8], f32, tag="stage")
    onesC = p_const.tile([128, 2], f32r, tag="onesC")      # 1/C for mean matmuls
    nc.vector.memset(stage[:, 0:2], 1.0 / C)
    nc.vector.tensor_copy(onesC[:], stage[:, 0:2])
    stage_row = p_const.tile([1, 512], f32, tag="stage_row")
    nc.vector.memset(stage_row[:, 0:128], 1.0)
    onescol = p_const.tile([1, 128], f32r, tag="onescol")  # bcast lhsT
    nc.vector.tensor_copy(onescol[:], stage_row[:, 0:128])
    onesrow = p_const.tile([1, 512], bf16, tag="onesrow")  # moving row for bias
    nc.vector.memset(onesrow[:], 1.0)
    eps_t = p_const.tile([1, 1], f32, tag="eps")
    nc.vector.memset(eps_t, EPS)
    m01 = p_const.tile([128, 512], bf16, tag="m01")
    nc.sync.dma_start(out=m01, in_=m01_d)
    blm_t = p_const.tile([V, 1], f32, tag="blm")
    nc.sync.dma_start(out=blm_t, in_=blm_d.rearrange("(v o) -> v o", o=1))
    wlm_t = [p_const.tile([128, V], bf16, tag=f"wlm{kc}", name=f"wlm{kc}")
             for kc in range(KC)]
    for kc in range(KC):
        nc.sync.dma_start(out=wlm_t[kc], in_=wlm_d[kc * 128:(kc + 1) * 128, :])

    # ---- residual stream: one [128, 3*512] f32 tile per batch-pair ----
    x_t = [p_x.tile([128, KC * 512], f32r, tag=f"x{nt}", name=f"x{nt}")
           for nt in range(NBP)]
    for nt in range(NBP):
        for kc in range(KC):
            nc.sync.dma_start(out=x_t[nt][:, kc * 512:(kc + 1) * 512],
                              in_=x0T_d[kc * 128:(kc + 1) * 128,
                                        nt * 512:nt * 512 + 512])

    # ---- V_ext buffers: ones column written once, V slices per layer ----
    vext = [[p_v.tile([128, 2 * H * (HS + 1)], bf16, tag=f"vext{nt}_{bi}",
                      name=f"vext{nt}_{bi}") for bi in range(2)]
            for nt in range(NBP)]
    for nt in range(NBP):
        for bi in range(2):
            vxr = vext[nt][bi].rearrange("p (j h e) -> p j h e", j=2, h=H)
            nc.vector.memset(vxr[:, :, :, HS:HS + 1], 1.0)

    weights = {}

    def load_wqkv(l):
        w = weights.setdefault(l, {})
        w["wqkv"] = [p_w.tile([128, 3 * C], bf16, tag=f"wqkv{kc}",
                              name=f"wqkv{kc}", bufs=2) for kc in range(KC)]
        for kc in range(KC):
            nc.sync.dma_start(out=w["wqkv"][kc],
                              in_=wqkv_d[l, kc * 128:(kc + 1) * 128, :])
        w["bqkv"] = p_w.tile([128, 6], f32, tag="bqkv", name="bqkv", bufs=2)
        nc.sync.dma_start(out=w["bqkv"], in_=bqkv_d[l])

    def load_rest(l):
        w = weights.setdefault(l, {})
        w["wo"] = [p_w.tile([128, C], bf16, tag=f"wo{kc}", name=f"wo{kc}",
                            bufs=2) for kc in range(KC)]
        for kc in range(KC):
            nc.sync.dma_start(out=w["wo"][kc],
                              in_=wo_d[l, kc * 128:(kc + 1) * 128, :])
        w["w1"] = [p_w.tile([128, DFF], bf16, tag=f"w1{kc}", name=f"w1{kc}",
                            bufs=2) for kc in range(KC)]
        for kc in range(KC):
            nc.sync.dma_start(out=w["w1"][kc],
                              in_=w1_d[l, kc * 128:(kc + 1) * 128, :])
        w["b1"] = p_w.tile([128, K12], f32, tag="b1", name="b1", bufs=2)
        nc.sync.dma_start(out=w["b1"], in_=b1_d[l])
        w["w2"] = [p_w.tile([128, C], bf16, tag=f"w2_{kc}", name=f"w2_{kc}",
                            bufs=2) for kc in range(K12)]
        for kc in range(K12):
            nc.sync.dma_start(out=w["w2"][kc],
                              in_=w2_d[l, kc * 128:(kc + 1) * 128, :])
        w["brows"] = p_w.tile([1, 2 * C], bf16, tag="brows", name="brows",
                              bufs=2)
        nc.sync.dma_start(out=w["brows"], in_=brows_d[l])

    def stage_LN(nt, tag):
        """Standardize x_t[nt] -> new [128, 1536] bf16 tile.

        Stats via ones-matmuls; rsqrt as exp(-0.5*ln(var+eps)) so the whole
        kernel stays inside one ACT table set; per-token broadcasts via
        ones-column matmuls on the PE.
        """
        x = x_t[nt]
        xsq = p_big.tile([128, KC * 512], f32r, tag="xbig", name="xsq", bufs=2)
        nc.vector.tensor_mul(xsq[:], x[:], x[:])
        mu_ps = ps_aux.tile([2, 512], f32, tag="aux", name="mu_ps")
        for kc in range(KC):
            nc.tensor.matmul(mu_ps[:], onesC[:],
                             x[:, kc * 512:(kc + 1) * 512],
                             start=(kc == 0), stop=(kc == KC - 1))
        sq_ps = ps_aux.tile([2, 512], f32, tag="aux", name="sq_ps")
        for kc in range(KC):
            nc.tensor.matmul(sq_ps[:], onesC[:],
                             xsq[:, kc * 512:(kc + 1) * 512],
                             start=(kc == 0), stop=(kc == KC - 1))
        musq = p_rows.tile([1, 512], f32r, tag="musq", name="musq", bufs=2)
        nc.scalar.activation(musq[:], mu_ps[0:1, :], AF.Square, bias=0.0,
                             scale=1.0)
        var = p_rows.tile([1, 512], f32r, tag="var", name="var", bufs=2)
        nc.vector.tensor_tensor(out=var[:], in0=sq_ps[0:1, :], in1=musq[:],
                                op=ALU.subtract)
        mu_sb = p_rows.tile([1, 512], f32r, tag="mu_sb", name="mu_sb", bufs=2)
        nc.scalar.copy(mu_sb[:], mu_ps[0:1, :])
        lnv = p_rows.tile([1, 512], f32r, tag="lnv", name="lnv", bufs=2)
        nc.scalar.activation(lnv[:], var[:], AF.Ln, bias=eps_t[:], scale=1.0)
        rs = p_rows.tile([1, 512], f32r, tag="rs", name="rs", bufs=2)
        nc.scalar.activation(rs[:], lnv[:], AF.Exp, bias=0.0, scale=-0.5)
        mu_b = ps_aux.tile([128, 512], f32, tag="aux", name="mu_b")
        nc.tensor.matmul(mu_b[:], onescol[:], mu_sb[:], start=True, stop=True)
        rs_b = ps_aux.tile([128, 512], f32, tag="aux", name="rs_b")
        nc.tensor.matmul(rs_b[:], onescol[:], rs[:], start=True, stop=True)
        xn = p_xn.tile([128, KC * 512], bf16, tag=f"xn{nt}", name=f"xn{nt}",
                       bufs=1)
        for kc in range(KC):
            sl = slice(kc * 512, (kc + 1) * 512)
            xc = p_big.tile([128, 512], f32r, tag="xc", name="xc", bufs=2)
            nc.vector.tensor_tensor(out=xc[:], in0=x[:, sl], in1=mu_b[:],
                                    op=ALU.subtract)
            nc.vector.tensor_mul(xn[:, sl], xc[:], rs_b[:])
        return xn

    state = {}

    def stage_A(l, nt):
        state[nt] = {"xn": stage_LN(nt, "xn")}

    def stage_B(l, nt):
        w = weights[l]
        xn = state[nt]["xn"]
        qk = []
        for oc in range(6):
            qp = ps_big.tile([128, 512], f32, tag="big", name="qp")
            for kc in range(KC):
                nc.tensor.matmul(qp[:], w["wqkv"][kc][:, oc * 128:oc * 128 + 128],
                                 xn[:, kc * 512:(kc + 1) * 512],
                                 start=(kc == 0), stop=(kc == KC - 1))
            qt = p_qk.tile([128, 512], bf16, tag=f"qk{oc}", name=f"qk{oc}",
                           bufs=2)
            nc.scalar.activation(qt[:], qp[:], AF.Identity,
                                 bias=w["bqkv"][:, oc:oc + 1], scale=1.0)
            qk.append(qt)
        for bi in range(2):
            vxr = vext[nt][bi].rearrange("p (j h e) -> p j h e", j=2, h=H)
            for j in range(2):
                vp = ps_big.tile([128, C], f32, tag="big", name="vp")
                tc0 = bi * 256 + j * 128
                for kc in range(KC):
                    nc.tensor.matmul(vp[:], xn[:, kc * 512 + tc0:kc * 512 + tc0 + 128],
                                     w["wqkv"][kc][:, 2 * C:3 * C],
                                     start=(kc == 0), stop=(kc == KC - 1))
                nc.vector.tensor_copy(vxr[:, j, :, 0:HS],
                                      vp[:].rearrange("p (h d) -> p h d", h=H))
        state[nt]["qk"] = qk

    def stage_CD(l, nt):
        """Wave-pipelined scores -> exp -> mask -> attV -> normalize."""
        st = state[nt]
        qk = st["qk"]
        attc = [p_attc.tile([128, 512], bf16, tag=f"attc{kc}",
                            name=f"attc{kc}", bufs=3) for kc in range(KC)]
        ap_t = {}
        e_ms = {}
        LAG = 2
        for u in range(12 + LAG):
            if u < 12:
                h, bi = divmod(u, 2)
                qch, kch = h // 2, 3 + h // 2
                qrow = (h % 2) * 64
                q0 = bi * 256
                sp = ps_big.tile([128, 512], f32, tag="big", name="sp")
                qs = qk[qch][qrow:qrow + 64, q0:q0 + 256]
                nc.tensor.matmul(sp[:, 0:256],
                                 qk[kch][qrow:qrow + 64, q0:q0 + 128],
                                 qs, start=True, stop=True)
                nc.tensor.matmul(sp[:, 256:512],
                                 qk[kch][qrow:qrow + 64, q0 + 128:q0 + 256],
                                 qs, start=True, stop=True)
                e_t = p_e.tile([128, 512], bf16, tag="e_t", name="e_t", bufs=3)
                nc.scalar.activation(e_t[:], sp[:], AF.Exp, bias=0.0,
                                     scale=SCALE)
                e_m = p_e.tile([128, 512], bf16, tag="e_m", name="e_m", bufs=4)
                nc.vector.tensor_mul(e_m[:], e_t[:], m01[:])
                e_ms[u] = e_m
            if u >= LAG:
                v = u - LAG
                h, bi = divmod(v, 2)
                qch = h // 2
                qrow = (h % 2) * 64
                q0 = bi * 256
                if bi == 0:
                    ap_t[h] = ps_ap.tile([HS + 1, 512], f32, tag="ap",
                                         name="ap_")
                ap_ = ap_t[h]
                e_m = e_ms.pop(v)
                vxr = vext[nt][bi].rearrange("p (j h e) -> p j h e", j=2, h=H)
                nc.tensor.matmul(ap_[:, q0:q0 + 256], vxr[:, 0, h, :],
                                 e_m[:, 0:256], start=True, stop=False)
                nc.tensor.matmul(ap_[:, q0:q0 + 256], vxr[:, 1, h, :],
                                 e_m[:, 256:512], start=False, stop=True)
                if bi == 1:
                    # 1/sum = exp(-ln(sum)), broadcast over 64 partitions via
                    # a K=1 ones-matmul, then one DVE multiply per head.
                    lns = p_rows.tile([1, 512], f32r, tag="lns", name="lns",
                                      bufs=2)
                    nc.scalar.activation(lns[:], ap_[HS:HS + 1, :], AF.Ln,
                                         bias=0.0, scale=1.0)
                    rec = p_rows.tile([1, 512], f32r, tag="rec", name="rec",
                                      bufs=2)
                    nc.scalar.activation(rec[:], lns[:], AF.Exp, bias=0.0,
                                         scale=-1.0)
                    rec_b = ps_aux.tile([64, 512], f32, tag="aux",
                                        name="rec_b")
                    nc.tensor.matmul(rec_b[:], onescol[:, 0:64], rec[:],
                                     start=True, stop=True)
                    rb_sb = p_rows.tile([64, 512], bf16, tag="rb_sb",
                                        name="rb_sb", bufs=2)
                    nc.scalar.copy(rb_sb[:], rec_b[:])
                    qr2 = (h % 2) * 64
                    nc.vector.tensor_mul(attc[qch][qr2:qr2 + 64, :],
                                         ap_[0:HS, :], rb_sb[:])
        state[nt]["attc"] = attc
        del state[nt]["qk"], state[nt]["xn"]

    def stage_E(l, nt):
        w = weights[l]
        attc = state[nt]["attc"]
        for oc in range(KC):
            wp = ps_big.tile([128, 512], f32, tag="big", name="wp")
            nc.tensor.matmul(wp[:], w["brows"][0:1, oc * 128:oc * 128 + 128],
                             onesrow[:], start=True, stop=False)
            for kc in range(KC):
                nc.tensor.matmul(wp[:], w["wo"][kc][:, oc * 128:oc * 128 + 128],
                                 attc[kc][:], start=False, stop=(kc == KC - 1))
            sl = slice(oc * 512, (oc + 1) * 512)
            nc.vector.tensor_tensor(out=x_t[nt][:, sl], in0=wp[:],
                                    in1=x_t[nt][:, sl], op=ALU.add)
        del state[nt]["attc"]

    def stage_F(l, nt):
        state[nt]["h2n"] = stage_LN(nt, "h2n")

    def stage_G(l, nt):
        w = weights[l]
        h2n = state[nt]["h2n"]
        a_t = []
        for kc12 in range(K12):
            fp1 = ps_big.tile([128, 512], f32, tag="big", name="fp1")
            for kc in range(KC):
                nc.tensor.matmul(fp1[:],
                                 w["w1"][kc][:, kc12 * 128:kc12 * 128 + 128],
                                 h2n[:, kc * 512:(kc + 1) * 512],
                                 start=(kc == 0), stop=(kc == KC - 1))
            a = p_a.tile([128, 512], bf16, tag=f"a{kc12}", name=f"a{kc12}",
                         bufs=1)
            nc.scalar.activation(a[:], fp1[:], AF.Relu,
                                 bias=w["b1"][:, kc12:kc12 + 1], scale=1.0)
            a_t.append(a)
        state[nt]["a"] = a_t
        del state[nt]["h2n"]

    def stage_H(l, nt):
        w = weights[l]
        a_t = state[nt]["a"]
        for oc in range(KC):
            fp2 = ps_big.tile([128, 512], f32, tag="big", name="fp2")
            nc.tensor.matmul(fp2[:],
                             w["brows"][0:1, C + oc * 128:C + oc * 128 + 128],
                             onesrow[:], start=True, stop=False)
            for kc12 in range(K12):
                nc.tensor.matmul(fp2[:],
                                 w["w2"][kc12][:, oc * 128:oc * 128 + 128],
                                 a_t[kc12][:], start=False,
                                 stop=(kc12 == K12 - 1))
            sl = slice(oc * 512, (oc + 1) * 512)
            nc.vector.tensor_tensor(out=x_t[nt][:, sl], in0=fp2[:],
                                    in1=x_t[nt][:, sl], op=ALU.add)
        del state[nt]

    def stage_HEAD(nt):
        xf = stage_LN(nt, "xf")
        lp = ps_big.tile([V, 512], f32, tag="big", name="lp")
        for kc in range(KC):
            nc.tensor.matmul(lp[:], wlm_t[kc][:],
                             xf[:, kc * 512:(kc + 1) * 512],
                             start=(kc == 0), stop=(kc == KC - 1))
        osb = p_out.tile([V, 512], f32, tag="osb", name="osb")
        nc.scalar.activation(osb[:], lp[:], AF.Identity, bias=blm_t[:],
                             scale=1.0)
        nc.sync.dma_start(out=outT_d[:, nt * 512:nt * 512 + 512], in_=osb[:])

    # ---- stage-major emission: 4 independent batch-pair streams per stage ----
    load_wqkv(0)
    load_rest(0)
    for l in range(N_LAYERS):
        for nt in range(NBP):
            stage_A(l, nt)
        for nt in range(NBP):
            stage_B(l, nt)
            stage_CD(l, nt)
        if l + 1 < N_LAYERS:
            load_wqkv(l + 1)
        for nt in range(NBP):
            stage_E(l, nt)
        for nt in range(NBP):
            stage_F(l, nt)
        if l + 1 < N_LAYERS:
            load_rest(l + 1)
        for nt in range(NBP):
            stage_G(l, nt)
            stage_H(l, nt)

    for nt in range(NBP):
        stage_HEAD(nt)

    ctx.close()


def _host_prep(inputs):
    """Fold LN affine params into weights; build per-core input maps."""
    f = lambda k: np.asarray(inputs[k], dtype=np.float32)
    tobf = lambda a: np.ascontiguousarray(a.astype(ml_dtypes.bfloat16))
    idx = np.asarray(inputs["idx"]).astype(np.int64)
    tok_emb, pos_emb = f("tok_emb"), f("pos_emb")
    Wq, Wk, Wv, Wo = f("Wq"), f("Wk"), f("Wv"), f("Wo")
    bo, W1, b1, W2, b2 = f("bo"), f("W1"), f("b1"), f("W2"), f("b2")
    ln1_g, ln1_b = f("ln1_g"), f("ln1_b")
    ln2_g, ln2_b = f("ln2_g"), f("ln2_b")
    lnf_g, lnf_b = f("lnf_g"), f("lnf_b")
    Wlm, blm = f("Wlm"), f("blm")

    # [L,H,C,HS] -> [L,C,H*HS]
    Wq_all = np.transpose(Wq, (0, 2, 1, 3)).reshape(L, C, C)
    Wk_all = np.transpose(Wk, (0, 2, 1, 3)).reshape(L, C, C)
    Wv_all = np.transpose(Wv, (0, 2, 1, 3)).reshape(L, C, C)

    g1 = ln1_g[:, :, None]
    wqkv = np.concatenate([g1 * Wq_all, g1 * Wk_all, g1 * Wv_all], axis=2)
    bq = np.einsum("lc,lcd->ld", ln1_b, Wq_all)
    bk = np.einsum("lc,lcd->ld", ln1_b, Wk_all)
    bv = np.einsum("lc,lcd->ld", ln1_b, Wv_all)
    bo2 = bo + np.einsum("ld,ldc->lc", bv, Wo)       # v-bias folds through Wo
    w1f = ln2_g[:, :, None] * W1
    b1f = b1 + np.einsum("lc,lcd->ld", ln2_b, W1)
    wlmf = lnf_g[:, None] * Wlm
    blmf = blm + lnf_b @ Wlm

    bqkv = np.concatenate([bq, bk], axis=1)          # [L, 768]
    bqkv_cols = np.ascontiguousarray(
        bqkv.reshape(L, 6, 128).transpose(0, 2, 1)).astype(np.float32)
    b1_cols = np.ascontiguousarray(
        b1f.reshape(L, K12, 128).transpose(0, 2, 1)).astype(np.float32)
    brows = tobf(np.concatenate([bo2, b2], axis=1)[:, None, :])  # [L,1,2C]

    # multiplicative causal mask, key-major: cols = (key_block, q)
    p = np.arange(128)[:, None]
    q = np.arange(256)[None, :]
    m0 = (p <= q).astype(np.float32)          # keys 0..127
    m1 = (p + 128 <= q).astype(np.float32)    # keys 128..255
    m01 = tobf(np.concatenate([m0, m1], axis=1))    # [128, 512]

    x0 = tok_emb[idx] + pos_emb[None]                # [B,T,C] f32
    in_maps = []
    for c in range(N_CORES):
        x0c = x0[c * BPC:(c + 1) * BPC].reshape(NTOK, C)
        in_maps.append({
            "x0T": np.ascontiguousarray(x0c.T),
            "wqkv": tobf(wqkv),
            "bqkv": bqkv_cols,
            "wo": tobf(Wo),
            "w1": tobf(w1f),
            "b1": b1_cols,
            "w2": tobf(W2),
            "brows": brows,
            "wlm": tobf(wlmf),
            "blm": np.ascontiguousarray(blmf),
            "m01": m01,
        })
    return in_maps


def _run(inputs, trace=False):
    if "nc" not in _cache:
        _cache["nc"] = _build_nc()
    nc = _cache["nc"]
    in_maps = _host_prep(inputs)
    res = run_bass_kernel_spmd(nc, in_maps, core_ids=list(range(N_CORES)),
                               trace=trace)
    outs = []
    for c in range(N_CORES):
        outT = res.results[c]["outT"]                 # [V, NTOK]
        outs.append(outT.T.reshape(BPC, T, V))
    logits = np.concatenate(outs, axis=0).astype(np.float32)
    return logits, res


def kernel(**inputs) -> np.ndarray:
    logits, _ = _run(inputs, trace=False)
    return logits


# revision 17
# speedup vs baseline: 2.9745x; 1.4537x over previous
"""Bass/Trainium2 kernel for nn_CharLevelLanguageModel (6-layer char transformer).

v2 strategy: data-parallel over batch (64 -> 8 cores x 8). Per core, each layer
is emitted stage-major across the 4 batch-pair (512-token) tiles so that every
serial chain (LN row ops, softmax normalize) is covered by independent matmul
work from the other batch-pairs.

Key choices vs v1:
- bf16 matmul operands (weights + activations); residual stream and stats stay
  fp32 (f32r) in SBUF; PSUM accumulates fp32.
- Zero GpSimd instructions. Partition broadcasts are ones-column matmuls on the
  PE; elementwise work is split between DVE and ACT.
- Single ACT table set ("natural_log_exp_and_others"): rsqrt(v)=exp(-0.5*ln v),
  1/s = exp(-ln s). No table reloads after startup.
- LN affine params folded into adjacent weights on host; biases ride K=1
  bf16 matmul rows (brows x onesrow) or ACT bias columns.
- Causal mask is one multiplicative bf16 DVE op per (head, batch) unit.
"""

import os
import numpy as np
import ml_dtypes

import concourse.bass as bass
import concourse.mybir as mybir
import concourse.tile as tile
from concourse import bacc
from concourse.bass_utils import run_bass_kernel_spmd

B, T, C, H, L, V = 64, 256, 384, 6, 6, 65
HS = C // H          # 64
DFF = 4 * C          # 1536
N_CORES = 8
BPC = B // N_CORES   # 8 batches per core
NTOK = BPC * T       # 2048 tokens per core
NBP = 4              # batch-pair (512-token) tiles per core
KC = C // 128        # 3 feature chunks
K12 = DFF // 128     # 12 dff chunks
EPS = 1e-5
SCALE = HS ** -0.5

f32 = mybir.dt.float32
f32r = mybir.dt.float32r
bf16 = mybir.dt.bfloat16
i32 = mybir.dt.int32
AF = mybir.ActivationFunctionType
ALU = mybir.AluOpType

N_LAYERS = int(os.environ.get("KERNEL_LAYERS", str(L)))

_cache = {}


def _build_nc():
    nc = bacc.Bacc("TRN2", target_bir_lowering=False, debug=False,
                   num_devices=N_CORES)

    x0T_d = nc.dram_tensor("x0T", [C, NTOK], f32r, kind="ExternalInput").ap()
    wqkv_d = nc.dram_tensor("wqkv", [L, C, 3 * C], bf16, kind="ExternalInput").ap()
    bqkv_d = nc.dram_tensor("bqkv", [L, 128, 6], f32, kind="ExternalInput").ap()
    wo_d = nc.dram_tensor("wo", [L, C, C], bf16, kind="ExternalInput").ap()
    w1_d = nc.dram_tensor("w1", [L, C, DFF], bf16, kind="ExternalInput").ap()
    b1_d = nc.dram_tensor("b1", [L, 128, K12], f32, kind="ExternalInput").ap()
    w2_d = nc.dram_tensor("w2", [L, DFF, C], bf16, kind="ExternalInput").ap()
    brows_d = nc.dram_tensor("brows", [L, 1, 2 * C], bf16, kind="ExternalInput").ap()
    wlm_d = nc.dram_tensor("wlm", [C, V], bf16, kind="ExternalInput").ap()
    blm_d = nc.dram_tensor("blm", [V], f32, kind="ExternalInput").ap()
    m01_d = nc.dram_tensor("m01", [128, 512], bf16, kind="ExternalInput").ap()
    outT_d = nc.dram_tensor("outT", [V, NTOK], f32, kind="ExternalOutput").ap()

    with tile.TileContext(nc) as tc:
        _build_body(nc, tc, x0T_d, wqkv_d, bqkv_d, wo_d, w1_d, b1_d, w2_d,
                    brows_d, wlm_d, blm_d, m01_d, outT_d)
    nc.compile()
    return nc


def _build_body(nc, tc, x0T_d, wqkv_d, bqkv_d, wo_d, w1_d, b1_d, w2_d,
                brows_d, wlm_d, blm_d, m01_d, outT_d):
    import contextlib
    ctx = contextlib.ExitStack()
    p_const = ctx.enter_context(tc.tile_pool(name="consts", bufs=1))
    p_x = ctx.enter_context(tc.tile_pool(name="x", bufs=1))
    p_w = ctx.enter_context(tc.tile_pool(name="w", bufs=1))
    p_big = ctx.enter_context(tc.tile_pool(name="bigsb", bufs=1))   # xsq/xc
    p_xn = ctx.enter_context(tc.tile_pool(name="xn", bufs=1))
    p_rows = ctx.enter_context(tc.tile_pool(name="rows", bufs=1))
    p_qk = ctx.enter_context(tc.tile_pool(name="qk", bufs=1))
    p_v = ctx.enter_context(tc.tile_pool(name="v", bufs=1))
    p_e = ctx.enter_context(tc.tile_pool(name="e", bufs=1))
    p_attc = ctx.enter_context(tc.tile_pool(name="attc", bufs=1))
    p_a = ctx.enter_context(tc.tile_pool(name="a", bufs=1))
    p_out = ctx.enter_context(tc.tile_pool(name="out", bufs=2))
    ps_aux = ctx.enter_context(tc.tile_pool(name="ps_aux", bufs=2, space="PSUM"))
    ps_ap = ctx.enter_context(tc.tile_pool(name="ps_ap", bufs=2, space="PSUM"))
    ps_big = ctx.enter_context(tc.tile_pool(name="ps_big", bufs=4, space="PSUM"))

    # ---- constants ----
    stage = p_const.tile([128, 8], f32, tag="stage")
    onesC = p_const.tile([128, 2], f32r, tag="onesC")      # 1/C for mean matmuls
    nc.vector.memset(stage[:, 0:2], 1.0 / C)
    nc.vector.tensor_copy(onesC[:], stage[:, 0:2])
    stage_row = p_const.tile([1, 512], f32, tag="stage_row")
    nc.vector.memset(stage_row[:, 0:128], 1.0)
    onescol = p_const.tile([1, 128], f32r, tag="onescol")  # bcast lhsT
    nc.vector.tensor_copy(onescol[:], stage_row[:, 0:128])
    onesrow = p_const.tile([1, 512], bf16, tag="onesrow")  # moving row for bias
    nc.vector.memset(onesrow[:], 1.0)
    ones512r = p_const.tile([1, 512], f32r, tag="ones512r")
    nc.vector.memset(stage_row[:], 1.0)
    nc.vector.tensor_copy(ones512r[:], stage_row[:])
    epscol2 = p_const.tile([1, 2], f32r, tag="epscol2")    # eps rides sq-stats MM
    nc.vector.memset(stage[:, 2:4], EPS)
    nc.vector.tensor_copy(epscol2[:], stage[0:1, 2:4])
    m01 = p_const.tile([128, 512], bf16, tag="m01")
    nc.sync.dma_start(out=m01, in_=m01_d)
    blm_t = p_const.tile([V, 1], f32, tag="blm")
    nc.sync.dma_start(out=blm_t, in_=blm_d.rearrange("(v o) -> v o", o=1))
    wlm_t = [p_const.tile([128, V], bf16, tag=f"wlm{kc}", name=f"wlm{kc}")
             for kc in range(KC)]
    for kc in range(KC):
        nc.sync.dma_start(out=wlm_t[kc], in_=wlm_d[kc * 128:(kc + 1) * 128, :])

    # ---- residual stream: one [128, 3*512] f32 tile per batch-pair ----
    x_t = [p_x.tile([128, KC * 512], f32r, tag=f"x{nt}", name=f"x{nt}")
           for nt in range(NBP)]
    for nt in range(NBP):
        for kc in range(KC):
            nc.sync.dma_start(out=x_t[nt][:, kc * 512:(kc + 1) * 512],
                              in_=x0T_d[kc * 128:(kc + 1) * 128,
                                        nt * 512:nt * 512 + 512])

    # ---- V_ext buffers: ones column written once, V slices per layer ----
    vext = [[p_v.tile([128, 2 * H * (HS + 1)], bf16, tag=f"vext{nt}_{bi}",
                      name=f"vext{nt}_{bi}") for bi in range(2)]
            for nt in range(NBP)]
    for nt in range(NBP):
        for bi in range(2):
            vxr = vext[nt][bi].rearrange("p (j h e) -> p j h e", j=2, h=H)
            nc.vector.memset(vxr[:, :, :, HS:HS + 1], 1.0)

    weights = {}

    def load_wqkv(l):
        w = weights.setdefault(l, {})
        w["wqkv"] = [p_w.tile([128, 3 * C], bf16, tag=f"wqkv{kc}",
                              name=f"wqkv{kc}", bufs=2) for kc in range(KC)]
        for kc in range(KC):
            nc.sync.dma_start(out=w["wqkv"][kc],
                              in_=wqkv_d[l, kc * 128:(kc + 1) * 128, :])
        w["bqkv"] = p_w.tile([128, 6], f32, tag="bqkv", name="bqkv", bufs=2)
        nc.sync.dma_start(out=w["bqkv"], in_=bqkv_d[l])

    def load_rest(l):
        w = weights.setdefault(l, {})
        w["wo"] = [p_w.tile([128, C], bf16, tag=f"wo{kc}", name=f"wo{kc}",
                            bufs=2) for kc in range(KC)]
        for kc in range(KC):
            nc.sync.dma_start(out=w["wo"][kc],
                              in_=wo_d[l, kc * 128:(kc + 1) * 128, :])
        w["w1"] = [p_w.tile([128, DFF], bf16, tag=f"w1{kc}", name=f"w1{kc}",
                            bufs=2) for kc in range(KC)]
        for kc in range(KC):
            nc.sync.dma_start(out=w["w1"][kc],
                              in_=w1_d[l, kc * 128:(kc + 1) * 128, :])
        w["b1"] = p_w.tile([128, K12], f32, tag="b1", name="b1", bufs=2)
        nc.sync.dma_start(out=w["b1"], in_=b1_d[l])
        w["w2"] = [p_w.tile([128, C], bf16, tag=f"w2_{kc}", name=f"w2_{kc}",
                            bufs=2) for kc in range(K12)]
        for kc in range(K12):
            nc.sync.dma_start(out=w["w2"][kc],
                              in_=w2_d[l, kc * 128:(kc + 1) * 128, :])
        w["brows"] = p_w.tile([1, 2 * C], bf16, tag="brows", name="brows",
                              bufs=2)
        nc.sync.dma_start(out=w["brows"], in_=brows_d[l])

    def stage_LN(nt, tag):
        """Standardize x_t[nt] -> new [128, 1536] bf16 tile.

        Stats via ones-matmuls; rsqrt as exp(-0.5*ln(var+eps)) so the whole
        kernel stays inside one ACT table set; per-token broadcasts via
        ones-column matmuls on the PE.
        """
        x = x_t[nt]
        xsq = p_big.tile([128, KC * 512], f32r, tag="xbig", name="xsq", bufs=2)
        nc.vector.tensor_mul(xsq[:], x[:], x[:])
        mu_ps = ps_aux.tile([2, 512], f32, tag="aux", name="mu_ps")
        for kc in range(KC):
            nc.tensor.matmul(mu_ps[:], onesC[:],
                             x[:, kc * 512:(kc + 1) * 512],
                             start=(kc == 0), stop=(kc == KC - 1))
        sq_ps = ps_aux.tile([2, 512], f32, tag="aux", name="sq_ps")
        for kc in range(KC):
            nc.tensor.matmul(sq_ps[:], onesC[:],
                             xsq[:, kc * 512:(kc + 1) * 512],
                             start=(kc == 0), stop=False)
        nc.tensor.matmul(sq_ps[:], epscol2[:], ones512r[:], start=False,
                         stop=True)
        musq = p_rows.tile([1, 512], f32r, tag="musq", name="musq", bufs=1)
        nc.scalar.activation(musq[:], mu_ps[0:1, :], AF.Square, bias=0.0,
                             scale=1.0)
        # ve = E[x^2] + eps - mu^2
        ve = p_rows.tile([1, 512], f32, tag="ve", name="ve", bufs=2)
        nc.vector.tensor_tensor(out=ve[:], in0=sq_ps[0:1, :], in1=musq[:],
                                op=ALU.subtract)
        mu_sb = p_rows.tile([1, 512], f32r, tag="mu_sb", name="mu_sb", bufs=1)
        nc.scalar.copy(mu_sb[:], mu_ps[0:1, :])
        # rs = rsqrt(ve): quake seed + 2 Newton iterations, all on DVE so the
        # ACT engine never leaves the exp table set.
        ish = p_rows.tile([1, 512], i32, tag="ish", name="ish", bufs=1)
        nc.vector.tensor_scalar(out=ish[:], in0=ve[:].bitcast(i32), scalar1=1,
                                scalar2=None, op0=ALU.logical_shift_right)
        sdi = p_rows.tile([1, 512], i32, tag="sdi", name="sdi", bufs=1)
        nc.vector.tensor_scalar(out=sdi[:], in0=ish[:],
                                scalar1=float(0x5F3759DF), scalar2=-1.0,
                                op0=ALU.subtract, op1=ALU.mult)
        y0 = sdi[:].bitcast(f32)
        y1 = None
        for it_n in range(2):
            yv = y0 if it_n == 0 else y1[:]
            nt_a = p_rows.tile([1, 512], f32, tag="nt", name="nt", bufs=2)
            nc.vector.tensor_mul(nt_a[:], ve[:], yv)
            nt_b = p_rows.tile([1, 512], f32, tag="nt", name="nt", bufs=2)
            nc.vector.tensor_mul(nt_b[:], nt_a[:], yv)
            nt_c = p_rows.tile([1, 512], f32, tag="nc", name="nc", bufs=1)
            nc.vector.tensor_scalar(out=nt_c[:], in0=nt_b[:], scalar1=-0.5,
                                    scalar2=1.5, op0=ALU.mult, op1=ALU.add)
            dt_o = f32 if it_n == 0 else f32r
            tg = "y1" if it_n == 0 else "rs"
            y_n = p_rows.tile([1, 512], dt_o, tag=tg, name=tg, bufs=1)
            nc.vector.tensor_mul(y_n[:], nt_c[:], yv)
            y1 = y_n
        rs = y1
        mu_b = ps_aux.tile([128, 512], f32, tag="aux", name="mu_b")
        nc.tensor.matmul(mu_b[:], onescol[:], mu_sb[:], start=True, stop=True)
        rs_b = ps_aux.tile([128, 512], f32, tag="aux", name="rs_b")
        nc.tensor.matmul(rs_b[:], onescol[:], rs[:], start=True, stop=True)
        xn = p_xn.tile([128, KC * 512], bf16, tag=f"xn{nt}", name=f"xn{nt}",
                       bufs=1)
        for kc in range(KC):
            sl = slice(kc * 512, (kc + 1) * 512)
            xc = p_big.tile([128, 512], f32r, tag="xc", name="xc", bufs=2)
            nc.vector.tensor_tensor(out=xc[:], in0=x[:, sl], in1=mu_b[:],
                                    op=ALU.subtract)
            nc.vector.tensor_mul(xn[:, sl], xc[:], rs_b[:])
        return xn

    state = {}

    def stage_A(l, nt):
        state[nt] = {"xn": stage_LN(nt, "xn")}

    def stage_B(l, nt):
        w = weights[l]
        xn = state[nt]["xn"]
        qk = []
        for oc in range(6):
            qp = ps_big.tile([128, 512], f32, tag="big", name="qp")
            for kc in range(KC):
                nc.tensor.matmul(qp[:], w["wqkv"][kc][:, oc * 128:oc * 128 + 128],
                                 xn[:, kc * 512:(kc + 1) * 512],
                                 start=(kc == 0), stop=(kc == KC - 1))
            qt = p_qk.tile([128, 512], bf16, tag=f"qk{oc}", name=f"qk{oc}",
                           bufs=2)
            nc.scalar.activation(qt[:], qp[:], AF.Identity,
                                 bias=w["bqkv"][:, oc:oc + 1], scale=1.0)
            qk.append(qt)
        for bi in range(2):
            vxr = vext[nt][bi].rearrange("p (j h e) -> p j h e", j=2, h=H)
            for j in range(2):
                vp = ps_big.tile([128, C], f32, tag="big", name="vp")
                tc0 = bi * 256 + j * 128
                for kc in range(KC):
                    nc.tensor.matmul(vp[:], xn[:, kc * 512 + tc0:kc * 512 + tc0 + 128],
                                     w["wqkv"][kc][:, 2 * C:3 * C],
                                     start=(kc == 0), stop=(kc == KC - 1))
                nc.vector.tensor_copy(vxr[:, j, :, 0:HS],
                                      vp[:].rearrange("p (h d) -> p h d", h=H))
        state[nt]["qk"] = qk

    def stage_CD(l, nt):
        """Wave-pipelined scores -> exp -> mask -> attV -> normalize."""
        st = state[nt]
        qk = st["qk"]
        attc = [p_attc.tile([128, 512], bf16, tag=f"attc{kc}",
                            name=f"attc{kc}", bufs=3) for kc in range(KC)]
        ap_t = {}
        e_ms = {}
        LAG = 2
        for u in range(12 + LAG):
            if u < 12:
                h, bi = divmod(u, 2)
                qch, kch = h // 2, 3 + h // 2
                qrow = (h % 2) * 64
                q0 = bi * 256
                sp = ps_big.tile([128, 512], f32, tag="big", name="sp")
                qs = qk[qch][qrow:qrow + 64, q0:q0 + 256]
                nc.tensor.matmul(sp[:, 0:256],
                                 qk[kch][qrow:qrow + 64, q0:q0 + 128],
                                 qs, start=True, stop=True)
                nc.tensor.matmul(sp[:, 256:512],
                                 qk[kch][qrow:qrow + 64, q0 + 128:q0 + 256],
                                 qs, start=True, stop=True)
                e_t = p_e.tile([128, 512], bf16, tag="e_t", name="e_t", bufs=3)
                nc.scalar.activation(e_t[:], sp[:], AF.Exp, bias=0.0,
                                     scale=SCALE)
                e_m = p_e.tile([128, 512], bf16, tag="e_m", name="e_m", bufs=4)
                nc.vector.tensor_mul(e_m[:], e_t[:], m01[:])
                e_ms[u] = e_m
            if u >= LAG:
                v = u - LAG
                h, bi = divmod(v, 2)
                qch = h // 2
                qrow = (h % 2) * 64
                q0 = bi * 256
                if bi == 0:
                    ap_t[h] = ps_ap.tile([HS + 1, 512], f32, tag="ap",
                                         name="ap_")
                ap_ = ap_t[h]
                e_m = e_ms.pop(v)
                vxr = vext[nt][bi].rearrange("p (j h e) -> p j h e", j=2, h=H)
                nc.tensor.matmul(ap_[:, q0:q0 + 256], vxr[:, 0, h, :],
                                 e_m[:, 0:256], start=True, stop=False)
                nc.tensor.matmul(ap_[:, q0:q0 + 256], vxr[:, 1, h, :],
                                 e_m[:, 256:512], start=False, stop=True)
                if bi == 1:
                    # 1/sum via DVE reciprocal, broadcast over 64 partitions
                    # via a K=1 ones-matmul, then one DVE multiply per head.
                    srow = p_rows.tile([1, 512], f32, tag="srow", name="srow",
                                       bufs=1)
                    nc.scalar.copy(srow[:], ap_[HS:HS + 1, :])
                    rec = p_rows.tile([1, 512], f32, tag="rec", name="rec",
                                      bufs=1)
                    nc.vector.reciprocal_approx_fast(out=rec[:], in_=srow[:])
                    rec_r = p_rows.tile([1, 512], f32r, tag="rec_r",
                                        name="rec_r", bufs=1)
                    nc.vector.tensor_copy(rec_r[:], rec[:])
                    rec_b = ps_aux.tile([64, 512], f32, tag="aux",
                                        name="rec_b")
                    nc.tensor.matmul(rec_b[:], onescol[:, 0:64], rec_r[:],
                                     start=True, stop=True)
                    rb_sb = p_rows.tile([64, 512], bf16, tag="rb_sb",
                                        name="rb_sb", bufs=2)
                    nc.scalar.copy(rb_sb[:], rec_b[:])
                    qr2 = (h % 2) * 64
                    nc.vector.tensor_mul(attc[qch][qr2:qr2 + 64, :],
                                         ap_[0:HS, :], rb_sb[:])
        state[nt]["attc"] = attc
        del state[nt]["qk"], state[nt]["xn"]

    def stage_E(l, nt):
        w = weights[l]
        attc = state[nt]["attc"]
        for oc in range(KC):
            wp = ps_big.tile([128, 512], f32, tag="big", name="wp")
            nc.tensor.matmul(wp[:], w["brows"][0:1, oc * 128:oc * 128 + 128],
                             onesrow[:], start=True, stop=False)
            for kc in range(KC):
                nc.tensor.matmul(wp[:], w["wo"][kc][:, oc * 128:oc * 128 + 128],
                                 attc[kc][:], start=False, stop=(kc == KC - 1))
            sl = slice(oc * 512, (oc + 1) * 512)
            nc.vector.tensor_tensor(out=x_t[nt][:, sl], in0=wp[:],
                                    in1=x_t[nt][:, sl], op=ALU.add)
        del state[nt]["attc"]

    def stage_F(l, nt):
        state[nt]["h2n"] = stage_LN(nt, "h2n")

    def stage_G(l, nt):
        w = weights[l]
        h2n = state[nt]["h2n"]
        a_t = []
        for kc12 in range(K12):
            fp1 = ps_big.tile([128, 512], f32, tag="big", name="fp1")
            for kc in range(KC):
                nc.tensor.matmul(fp1[:],
                                 w["w1"][kc][:, kc12 * 128:kc12 * 128 + 128],
                                 h2n[:, kc * 512:(kc + 1) * 512],
                                 start=(kc == 0), stop=(kc == KC - 1))
            a = p_a.tile([128, 512], bf16, tag=f"a{kc12}", name=f"a{kc12}",
                         bufs=1)
            nc.scalar.activation(a[:], fp1[:], AF.Relu,
                                 bias=w["b1"][:, kc12:kc12 + 1], scale=1.0)
            a_t.append(a)
        state[nt]["a"] = a_t
        del state[nt]["h2n"]

    def stage_H(l, nt):
        w = weights[l]
        a_t = state[nt]["a"]
        for oc in range(KC):
            fp2 = ps_big.tile([128, 512], f32, tag="big", name="fp2")
            nc.tensor.matmul(fp2[:],
                             w["brows"][0:1, C + oc * 128:C + oc * 128 + 128],
                             onesrow[:], start=True, stop=False)
            for kc12 in range(K12):
                nc.tensor.matmul(fp2[:],
                                 w["w2"][kc12][:, oc * 128:oc * 128 + 128],
                                 a_t[kc12][:], start=False,
                                 stop=(kc12 == K12 - 1))
            sl = slice(oc * 512, (oc + 1) * 512)
            nc.vector.tensor_tensor(out=x_t[nt][:, sl], in0=fp2[:],
                                    in1=x_t[nt][:, sl], op=ALU.add)
        del state[nt]

    def stage_HEAD(nt):
        xf = stage_LN(nt, "xf")
        lp = ps_big.tile([V, 512], f32, tag="big", name="lp")
        for kc in range(KC):
            nc.tensor.matmul(lp[:], wlm_t[kc][:],
                             xf[:, kc * 512:(kc + 1) * 512],
                             start=(kc == 0), stop=(kc == KC - 1))
        osb = p_out.tile([V, 512], f32, tag="osb", name="osb")
        nc.scalar.activation(osb[:], lp[:], AF.Identity, bias=blm_t[:],
                             scale=1.0)
        nc.sync.dma_start(out=outT_d[:, nt * 512:nt * 512 + 512], in_=osb[:])

    # ---- stage-major emission: 4 independent batch-pair streams per stage ----
    load_wqkv(0)
    load_rest(0)
    for l in range(N_LAYERS):
        for nt in range(NBP):
            stage_A(l, nt)
        for nt in range(NBP):
            stage_B(l, nt)
            stage_CD(l, nt)
        if l + 1 < N_LAYERS:
            load_wqkv(l + 1)
        for nt in range(NBP):
            stage_E(l, nt)
        for nt in range(NBP):
            stage_F(l, nt)
        if l + 1 < N_LAYERS:
            load_rest(l + 1)
        for nt in range(NBP):
            stage_G(l, nt)
            stage_H(l, nt)

    for nt in range(NBP):
        stage_HEAD(nt)

    ctx.close()


def _host_prep(inputs):
    """Fold LN affine params into weights; build per-core input maps."""
    f = lambda k: np.asarray(inputs[k], dtype=np.float32)
    tobf = lambda a: np.ascontiguousarray(a.astype(ml_dtypes.bfloat16))
    idx = np.asarray(inputs["idx"]).astype(np.int64)
    tok_emb, pos_emb = f("tok_emb"), f("pos_emb")
    Wq, Wk, Wv, Wo = f("Wq"), f("Wk"), f("Wv"), f("Wo")
    bo, W1, b1, W2, b2 = f("bo"), f("W1"), f("b1"), f("W2"), f("b2")
    ln1_g, ln1_b = f("ln1_g"), f("ln1_b")
    ln2_g, ln2_b = f("ln2_g"), f("ln2_b")
    lnf_g, lnf_b = f("lnf_g"), f("lnf_b")
    Wlm, blm = f("Wlm"), f("blm")

    # [L,H,C,HS] -> [L,C,H*HS]
    Wq_all = np.transpose(Wq, (0, 2, 1, 3)).reshape(L, C, C)
    Wk_all = np.transpose(Wk, (0, 2, 1, 3)).reshape(L, C, C)
    Wv_all = np.transpose(Wv, (0, 2, 1, 3)).reshape(L, C, C)

    g1 = ln1_g[:, :, None]
    wqkv = np.concatenate([g1 * Wq_all, g1 * Wk_all, g1 * Wv_all], axis=2)
    bq = np.einsum("lc,lcd->ld", ln1_b, Wq_all)
    bk = np.einsum("lc,lcd->ld", ln1_b, Wk_all)
    bv = np.einsum("lc,lcd->ld", ln1_b, Wv_all)
    bo2 = bo + np.einsum("ld,ldc->lc", bv, Wo)       # v-bias folds through Wo
    w1f = ln2_g[:, :, None] * W1
    b1f = b1 + np.einsum("lc,lcd->ld", ln2_b, W1)
    wlmf = lnf_g[:, None] * Wlm
    blmf = blm + lnf_b @ Wlm

    bqkv = np.concatenate([bq, bk], axis=1)          # [L, 768]
    bqkv_cols = np.ascontiguousarray(
        bqkv.reshape(L, 6, 128).transpose(0, 2, 1)).astype(np.float32)
    b1_cols = np.ascontiguousarray(
        b1f.reshape(L, K12, 128).transpose(0, 2, 1)).astype(np.float32)
    brows = tobf(np.concatenate([bo2, b2], axis=1)[:, None, :])  # [L,1,2C]

    # multiplicative causal mask, key-major: cols = (key_block, q)
    p = np.arange(128)[:, None]
    q = np.arange(256)[None, :]
    m0 = (p <= q).astype(np.float32)          # keys 0..127
    m1 = (p + 128 <= q).astype(np.float32)    # keys 128..255
    m01 = tobf(np.concatenate([m0, m1], axis=1))    # [128, 512]

    x0 = tok_emb[idx] + pos_emb[None]                # [B,T,C] f32
    in_maps = []
    for c in range(N_CORES):
        x0c = x0[c * BPC:(c + 1) * BPC].reshape(NTOK, C)
        in_maps.append({
            "x0T": np.ascontiguousarray(x0c.T),
            "wqkv": tobf(wqkv),
            "bqkv": bqkv_cols,
            "wo": tobf(Wo),
            "w1": tobf(w1f),
            "b1": b1_cols,
            "w2": tobf(W2),
            "brows": brows,
            "wlm": tobf(wlmf),
            "blm": np.ascontiguousarray(blmf),
            "m01": m01,
        })
    return in_maps


def _run(inputs, trace=False):
    if "nc" not in _cache:
        _cache["nc"] = _build_nc()
    nc = _cache["nc"]
    in_maps = _host_prep(inputs)
    res = run_bass_kernel_spmd(nc, in_maps, core_ids=list(range(N_CORES)),
                               trace=trace)
    outs = []
    for c in range(N_CORES):
        outT = res.results[c]["outT"]                 # [V, NTOK]
        outs.append(outT.T.reshape(BPC, T, V))
    logits = np.concatenate(outs, axis=0).astype(np.float32)
    return logits, res


def kernel(**inputs) -> np.ndarray:
    logits, _ = _run(inputs, trace=False)
    return logits
